# revision 16
# baseline (speedup 1.0000x reference)
"""Trainium2 Bass kernel for nn_Attention_52407190945839 (channel attention).

Two small SPMD launches over 8 cores, data parallel over (batch, 64-row
strips of H); no collectives (their per-process comm init costs tens of
seconds under the axon tunnel, far more than the extra host round trip):

  - Fuse the 1x1 qkv conv and the 3x3 depthwise conv into a dense 9-tap
    conv: W2[o,i,tap] = qkv_w[o,i] * dw_w[o,tap].
  - Kernel A (per core): q/k = conv(x) in flipped layout (positions on
    partitions) feeding Gram matmuls [q.q | q.k | k.k] accumulated in PSUM
    -> tiny [128, 384] per-core output.
  - Host: sum Gram partials per batch, run the 16x16-per-head attention
    math (normalize / softmax / gelu scale-shift) in numpy, and fold
    proj @ blockdiag(attn) into the v-conv weights:
    Wy_tap = (proj @ bd) @ Wv_tap.
  - Kernel B (per core): y = 9-tap conv(x) with the folded weights. The
    v tensor is never materialized anywhere.

All heavy I/O is bf16 (inputs cast on host, output cast back); a JAX
persistent compilation cache makes reruns skip the NEFF compile.
"""

import math
import os
from contextlib import ExitStack

import numpy as np
import ml_dtypes

BF16 = ml_dtypes.bfloat16

DIM = 128
HEADS = 8
C = DIM // HEADS       # 16
H = W = 256
B = 2
N_CORES = 8
ROWS = H // 4          # 64 rows per core
L_CORE = ROWS * W      # 16384 positions per core
NTILE = L_CORE // 512  # 32 tiles of 512 for output streaming
NCHUNK = L_CORE // 128  # 128 chunks of 128 positions for qk/gram

CACHE_DIR = "/root/.cache/bass_jax_cache"
BUILD_DIR = "/root/.cache/bass_kernel_build"
GROUPS = [[0, 1, 2, 3], [4, 5, 6, 7]]

LAST_TIMING = {}


# ---------------------------------------------------------------- host ----

def _build_host_tensors(x, qkv_w, dw_w):
    qkv2 = qkv_w[:, :, 0, 0]              # [384, 128]
    dw2 = dw_w[:, 0].reshape(3 * DIM, 9)  # [384, 9]

    # wqk[i, tap*256 + o] = qkv2[o, i] * dw2[o, tap]   (o in 0..255 = q|k)
    wqk = np.empty((DIM, 9 * 2 * DIM), dtype=np.float32)
    # wvr[vd, tap*128 + i] = qkv2[256 + vd, i] * dw2[256 + vd, tap]
    wvr = np.empty((DIM, 9 * DIM), dtype=np.float32)
    for tap in range(9):
        wqk[:, tap * 2 * DIM:(tap + 1) * 2 * DIM] = \
            (qkv2[:2 * DIM] * dw2[:2 * DIM, tap:tap + 1]).T
        wvr[:, tap * DIM:(tap + 1) * DIM] = \
            qkv2[2 * DIM:] * dw2[2 * DIM:, tap:tap + 1]

    xpads = []
    for core in range(N_CORES):
        b, quad = divmod(core, 4)
        r0 = quad * ROWS
        xp = np.zeros((DIM, ROWS + 2, W + 2), dtype=np.float32)
        xp[:, 1:ROWS + 1, 1:W + 1] = x[b, :, r0:r0 + ROWS, :]
        if r0 > 0:
            xp[:, 0, 1:W + 1] = x[b, :, r0 - 1, :]
        if r0 + ROWS < H:
            xp[:, ROWS + 1, 1:W + 1] = x[b, :, r0 + ROWS, :]
        xpads.append(xp.astype(BF16))
    return xpads, wqk.astype(BF16), wvr


def _attention_host(grams, wvr, proj_w, attca_w, temperature):
    """Per-batch: Gram -> attention math -> folded y-conv weights (bf16)."""
    f = np.float32
    attca2 = attca_w[:, :, 0, 0].astype(f)   # [32, 16]
    projT = proj_w[:, :, 0, 0].T.astype(f)   # [i, o]
    temp128 = np.repeat(temperature.reshape(HEADS).astype(f), C)[:, None]
    wvr_f = wvr.astype(f)

    wys = []
    for b in range(B):
        g = np.sum([grams[c] for c in GROUPS[b]], axis=0, dtype=np.float64)
        g = g.astype(np.float64)
        sq2 = np.diag(g[:, :DIM]).copy()
        sk2 = np.diag(g[:, 2 * DIM:]).copy()
        rq = 1.0 / np.maximum(np.sqrt(sq2), 1e-12)
        rk = 1.0 / np.maximum(np.sqrt(sk2), 1e-12)
        attnb0 = np.empty((DIM, C))
        rkb = np.empty((DIM, C))
        for h in range(HEADS):
            attnb0[h * C:(h + 1) * C] = g[h * C:(h + 1) * C,
                                          DIM + h * C:DIM + (h + 1) * C]
            rkb[h * C:(h + 1) * C] = rk[h * C:(h + 1) * C]
        attn = attnb0 * rkb * rq[:, None] * temp128
        m = attn.max(axis=1, keepdims=True)
        e = np.exp(attn - m)
        attn0 = e / e.sum(axis=1, keepdims=True)
        rl = np.maximum(attn, 0.0)
        r2 = rl * rl
        erf = np.vectorize(math.erf)
        gl = 0.5 * r2 * (1.0 + erf(r2 / math.sqrt(2.0)))
        a1 = gl * r2
        ss = a1 @ attca2.T  # [128, 32]
        attnf = attn0 * (1.0 + ss[:, :C]) + ss[:, C:]
        bd = np.zeros((DIM, DIM))
        for h in range(HEADS):
            bd[h * C:(h + 1) * C, h * C:(h + 1) * C] = attnf[h * C:(h + 1) * C]
        mt = (bd.T @ projT).astype(f)  # [vd, o] = M^T
        wy = np.empty((DIM, 9 * DIM), dtype=f)
        for tap in range(9):
            wy[:, tap * DIM:(tap + 1) * DIM] = \
                wvr_f[:, tap * DIM:(tap + 1) * DIM].T @ mt
        wys.append(wy.astype(BF16))
    return wys


# ---------------------------------------------------------------- device ----

def _build_kernel_a(bass, bacc, mybir, tile):
    """xpad, wqk -> per-core Gram partials [128, 384]."""
    nc = bacc.Bacc("TRN2", target_bir_lowering=False, debug=False,
                   num_devices=N_CORES)
    f32 = mybir.dt.float32
    bf16 = mybir.dt.bfloat16
    xpad = nc.dram_tensor("xpad", [DIM, ROWS + 2, W + 2], bf16, kind="ExternalInput").ap()
    wqk = nc.dram_tensor("wqk", [DIM, 9 * 2 * DIM], bf16, kind="ExternalInput").ap()
    gout = nc.dram_tensor("gout", [DIM, 3 * DIM], f32, kind="ExternalOutput").ap()

    with tile.TileContext(nc) as tc, ExitStack() as ctx:
        const = ctx.enter_context(tc.tile_pool(name="const", bufs=1))
        qkpool = ctx.enter_context(tc.tile_pool(name="qksb", bufs=4))
        gsb_pool = ctx.enter_context(tc.tile_pool(name="gsb", bufs=1))
        psqk = ctx.enter_context(tc.tile_pool(name="psqk", bufs=3, space="PSUM"))
        psg = ctx.enter_context(tc.tile_pool(name="psg", bufs=1, space="PSUM"))

        wqk_sb = const.tile([DIM, 9 * 2 * DIM], bf16)
        nc.sync.dma_start(wqk_sb[:], wqk)
        xsb = const.tile([DIM, ROWS + 2, W + 2], bf16)
        for lo, hi in [(0, 18), (18, 34), (34, 50), (50, ROWS + 2)]:
            nc.sync.dma_start(xsb[:, lo:hi, :], xpad[:, lo:hi, :])

        g1 = psg.tile([DIM, 2 * DIM], f32)   # q.q | q.k
        g2 = psg.tile([DIM, DIM], f32)       # k.k
        for ch in range(NCHUNK):
            r, wi = divmod(ch, 2)
            w0 = wi * 128
            pqk = psqk.tile([DIM, 2 * DIM], f32)
            for tap in range(9):
                dh, dw = divmod(tap, 3)
                nc.tensor.matmul(
                    pqk[:],
                    lhsT=xsb[:, r + dh, w0 + dw:w0 + dw + 128],
                    rhs=wqk_sb[:, tap * 2 * DIM:(tap + 1) * 2 * DIM],
                    start=(tap == 0), stop=(tap == 8),
                )
            qkt = qkpool.tile([DIM, 2 * DIM], f32)
            nc.vector.tensor_copy(out=qkt[:], in_=pqk[:])
            nc.tensor.matmul(g1[:], lhsT=qkt[:, :DIM], rhs=qkt[:],
                             start=(ch == 0), stop=(ch == NCHUNK - 1))
            nc.tensor.matmul(g2[:], lhsT=qkt[:, DIM:], rhs=qkt[:, DIM:],
                             start=(ch == 0), stop=(ch == NCHUNK - 1))

        gsb = gsb_pool.tile([DIM, 3 * DIM], f32)
        nc.vector.tensor_copy(out=gsb[:, :2 * DIM], in_=g1[:])
        nc.vector.tensor_copy(out=gsb[:, 2 * DIM:], in_=g2[:])
        nc.sync.dma_start(gout, gsb[:])
    nc.compile()
    return nc


def _build_kernel_b(bass, bacc, mybir, tile):
    """xpad, wy -> y via the folded 9-tap conv."""
    nc = bacc.Bacc("TRN2", target_bir_lowering=False, debug=False,
                   num_devices=N_CORES)
    bf16 = mybir.dt.bfloat16
    f32 = mybir.dt.float32
    xpad = nc.dram_tensor("xpad", [DIM, ROWS + 2, W + 2], bf16, kind="ExternalInput").ap()
    wy = nc.dram_tensor("wy", [DIM, 9 * DIM], bf16, kind="ExternalInput").ap()
    yout = nc.dram_tensor("yout", [DIM, L_CORE], bf16, kind="ExternalOutput").ap()

    with tile.TileContext(nc) as tc, ExitStack() as ctx:
        const = ctx.enter_context(tc.tile_pool(name="const", bufs=1))
        ypool = ctx.enter_context(tc.tile_pool(name="ysb", bufs=4))
        psy = ctx.enter_context(tc.tile_pool(name="psy", bufs=4, space="PSUM"))

        wy_sb = const.tile([DIM, 9 * DIM], bf16)
        nc.sync.dma_start(wy_sb[:], wy)
        xsb = const.tile([DIM, ROWS + 2, W + 2], bf16)
        for lo, hi in [(0, 18), (18, 34), (34, 50), (50, ROWS + 2)]:
            nc.sync.dma_start(xsb[:, lo:hi, :], xpad[:, lo:hi, :])

        for rp in range(NTILE):
            py = psy.tile([DIM, 512], f32)
            for tap in range(9):
                dh, dw = divmod(tap, 3)
                nc.tensor.matmul(
                    py[:],
                    lhsT=wy_sb[:, tap * DIM:(tap + 1) * DIM],
                    rhs=xsb[:, 2 * rp + dh:2 * rp + dh + 2, dw:dw + W],
                    start=(tap == 0), stop=(tap == 8),
                )
            y_sb = ypool.tile([DIM, 512], bf16)
            nc.vector.tensor_copy(out=y_sb[:], in_=py[:])
            nc.sync.dma_start(yout[:, rp * 512:(rp + 1) * 512], y_sb[:])
    nc.compile()
    return nc


def _builder_module():
    """Import this file's builders from a fixed path.

    The BIR embeds per-instruction debug info (source filename + line), so
    building from a varying directory changes the BIR bytes and misses the
    persistent compilation cache. Copying this file to a stable location
    and importing the builders from there keeps the cache key directory-
    independent.
    """
    import importlib.util
    import shutil

    os.makedirs(BUILD_DIR, exist_ok=True)
    fixed = os.path.join(BUILD_DIR, "bass_kernel_fixed.py")
    src = os.path.abspath(__file__)
    try:
        with open(src, "rb") as f:
            body = f.read()
        old = b""
        if os.path.exists(fixed):
            with open(fixed, "rb") as f:
                old = f.read()
        if old != body:
            shutil.copyfile(src, fixed)
        spec = importlib.util.spec_from_file_location("bass_kernel_fixed", fixed)
        mod = importlib.util.module_from_spec(spec)
        spec.loader.exec_module(mod)
        return mod
    except Exception:
        import sys
        return sys.modules[__name__]


def _run_device(xpads, wqk, wvr, proj_w, attca_w, temperature):
    os.makedirs(CACHE_DIR, exist_ok=True)
    os.environ.setdefault("JAX_COMPILATION_CACHE_DIR", CACHE_DIR)

    import jax
    jax.config.update("jax_compilation_cache_dir", CACHE_DIR)
    jax.config.update("jax_persistent_cache_min_entry_size_bytes", -1)
    jax.config.update("jax_persistent_cache_min_compile_time_secs", 0.0)

    import concourse.bass as bass
    import concourse.bacc as bacc
    import concourse.mybir as mybir
    import concourse.tile as tile
    from concourse import bass_utils

    import time as _time

    core_ids = list(range(N_CORES))
    bmod = _builder_module()

    t0 = _time.perf_counter()
    nc_a = bmod._build_kernel_a(bass, bacc, mybir, tile)
    t1 = _time.perf_counter()
    in_maps_a = [{"xpad": xpads[c], "wqk": wqk} for c in core_ids]
    res_a = bass_utils.run_bass_kernel_spmd(nc_a, in_maps_a, core_ids)
    t2 = _time.perf_counter()
    grams = [r["gout"] for r in res_a.results]

    wys = _attention_host(grams, wvr, proj_w, attca_w, temperature)

    t3 = _time.perf_counter()
    nc_b = bmod._build_kernel_b(bass, bacc, mybir, tile)
    t4 = _time.perf_counter()
    in_maps_b = [{"xpad": xpads[c], "wy": wys[c // 4]} for c in core_ids]
    res_b = bass_utils.run_bass_kernel_spmd(nc_b, in_maps_b, core_ids)
    t5 = _time.perf_counter()

    LAST_TIMING["build_a_s"] = t1 - t0
    LAST_TIMING["run_a_s"] = t2 - t1
    LAST_TIMING["host_s"] = t3 - t2
    LAST_TIMING["build_b_s"] = t4 - t3
    LAST_TIMING["run_b_s"] = t5 - t4
    LAST_TIMING["kernel_a_ns"] = res_a.exec_time_ns
    LAST_TIMING["kernel_b_ns"] = res_b.exec_time_ns
    return [r["yout"] for r in res_b.results]


# ------------------------------------------------------------- emulation ----

def _emulate_device(xpads, wqk, wvr, proj_w, attca_w, temperature):
    """Numpy re-implementation of both device kernels."""
    f = np.float32
    wqk_f = wqk.astype(f)
    grams = []
    for core in range(N_CORES):
        xp = xpads[core].astype(f)  # [128, 66, 258]
        qk = np.zeros((2 * DIM, ROWS, W), dtype=f)
        for tap in range(9):
            dh, dw = divmod(tap, 3)
            xs = xp[:, dh:dh + ROWS, dw:dw + W]
            qk += np.einsum('io,ihw->ohw',
                            wqk_f[:, tap * 2 * DIM:(tap + 1) * 2 * DIM], xs)
        q = qk[:DIM].reshape(DIM, L_CORE)
        k = qk[DIM:].reshape(DIM, L_CORE)
        g = np.empty((DIM, 3 * DIM), dtype=f)
        g[:, :DIM] = q @ q.T
        g[:, DIM:2 * DIM] = q @ k.T
        g[:, 2 * DIM:] = k @ k.T
        grams.append(g)

    wys = _attention_host(grams, wvr, proj_w, attca_w, temperature)

    youts = []
    for core in range(N_CORES):
        wy = wys[core // 4].astype(f)
        xp = xpads[core].astype(f)
        y = np.zeros((DIM, ROWS, W), dtype=f)
        for tap in range(9):
            dh, dw = divmod(tap, 3)
            xs = xp[:, dh:dh + ROWS, dw:dw + W]
            y += np.einsum('io,ihw->ohw', wy[:, tap * DIM:(tap + 1) * DIM], xs)
        youts.append(y.reshape(DIM, L_CORE).astype(BF16))
    return youts


# ---------------------------------------------------------------- entry ----

def kernel(x, qkv_w, dw_w, proj_w, attca_w, temperature):
    x = np.asarray(x, dtype=np.float32)
    qkv_w = np.asarray(qkv_w, dtype=np.float32)
    dw_w = np.asarray(dw_w, dtype=np.float32)
    proj_w = np.asarray(proj_w, dtype=np.float32)
    attca_w = np.asarray(attca_w, dtype=np.float32)
    temperature = np.asarray(temperature, dtype=np.float32)

    xpads, wqk, wvr = _build_host_tensors(x, qkv_w, dw_w)

    if os.environ.get("KERNEL_EMULATE", "0") == "1":
        youts = _emulate_device(xpads, wqk, wvr, proj_w, attca_w, temperature)
    else:
        youts = _run_device(xpads, wqk, wvr, proj_w, attca_w, temperature)

    out = np.empty((B, DIM, H, W), dtype=np.float32)
    for core in range(N_CORES):
        b, quad = divmod(core, 4)
        r0 = quad * ROWS
        out[b, :, r0:r0 + ROWS, :] = \
            np.asarray(youts[core]).astype(np.float32).reshape(DIM, ROWS, W)
    return out


# revision 20
# speedup vs baseline: 1.0416x; 1.0416x over previous
"""Trainium2 Bass kernel for nn_Attention_52407190945839 (channel attention).

Two small SPMD launches over 8 cores, data parallel over (batch, 64-row
strips of H); no collectives (their per-process comm init costs tens of
seconds under the axon tunnel, far more than the extra host round trip):

  - Fuse the 1x1 qkv conv and the 3x3 depthwise conv into a dense 9-tap
    conv: W2[o,i,tap] = qkv_w[o,i] * dw_w[o,tap].
  - Kernel A (per core): q/k = conv(x) in flipped layout (positions on
    partitions) feeding Gram matmuls [q.q | q.k | k.k] accumulated in PSUM
    -> tiny [128, 384] per-core output.
  - Host: sum Gram partials per batch, run the 16x16-per-head attention
    math (normalize / softmax / gelu scale-shift) in numpy, and fold
    proj @ blockdiag(attn) into the v-conv weights:
    Wy_tap = (proj @ bd) @ Wv_tap.
  - Kernel B (per core): y = 9-tap conv(x) with the folded weights. The
    v tensor is never materialized anywhere.

All heavy I/O is bf16 (inputs cast on host, output cast back); a JAX
persistent compilation cache makes reruns skip the NEFF compile.
"""

import math
import os
from contextlib import ExitStack

import numpy as np
import ml_dtypes

BF16 = ml_dtypes.bfloat16

DIM = 128
HEADS = 8
C = DIM // HEADS       # 16
H = W = 256
B = 2
N_CORES = 8
ROWS = H // 4          # 64 rows per core
L_CORE = ROWS * W      # 16384 positions per core
NTILE = L_CORE // 512  # 32 tiles of 512 for output streaming
NCHUNK = L_CORE // 128  # 128 chunks of 128 positions for qk/gram

CACHE_DIR = "/root/.cache/bass_jax_cache"
BUILD_DIR = "/root/.cache/bass_kernel_build"
GROUPS = [[0, 1, 2, 3], [4, 5, 6, 7]]

LAST_TIMING = {}


# ---------------------------------------------------------------- host ----

def _build_host_tensors(x, qkv_w, dw_w):
    qkv2 = qkv_w[:, :, 0, 0]              # [384, 128]
    dw2 = dw_w[:, 0].reshape(3 * DIM, 9)  # [384, 9]

    # wqk[i, tap*256 + o] = qkv2[o, i] * dw2[o, tap]   (o in 0..255 = q|k)
    wqk = np.empty((DIM, 9 * 2 * DIM), dtype=np.float32)
    # wvr[vd, tap*128 + i] = qkv2[256 + vd, i] * dw2[256 + vd, tap]
    wvr = np.empty((DIM, 9 * DIM), dtype=np.float32)
    for tap in range(9):
        wqk[:, tap * 2 * DIM:(tap + 1) * 2 * DIM] = \
            (qkv2[:2 * DIM] * dw2[:2 * DIM, tap:tap + 1]).T
        wvr[:, tap * DIM:(tap + 1) * DIM] = \
            qkv2[2 * DIM:] * dw2[2 * DIM:, tap:tap + 1]

    xpads = []
    for core in range(N_CORES):
        b, quad = divmod(core, 4)
        r0 = quad * ROWS
        xp = np.zeros((DIM, ROWS + 2, W + 2), dtype=np.float32)
        xp[:, 1:ROWS + 1, 1:W + 1] = x[b, :, r0:r0 + ROWS, :]
        if r0 > 0:
            xp[:, 0, 1:W + 1] = x[b, :, r0 - 1, :]
        if r0 + ROWS < H:
            xp[:, ROWS + 1, 1:W + 1] = x[b, :, r0 + ROWS, :]
        xpads.append(xp.astype(BF16))
    return xpads, wqk.astype(BF16), wvr


def _attention_host(grams, wvr, proj_w, attca_w, temperature):
    """Per-batch: Gram -> attention math -> folded y-conv weights (bf16)."""
    f = np.float32
    attca2 = attca_w[:, :, 0, 0].astype(f)   # [32, 16]
    projT = proj_w[:, :, 0, 0].T.astype(f)   # [i, o]
    temp128 = np.repeat(temperature.reshape(HEADS).astype(f), C)[:, None]
    wvr_f = wvr.astype(f)

    wys = []
    for b in range(B):
        g = np.sum([grams[c] for c in GROUPS[b]], axis=0, dtype=np.float64)
        g = g.astype(np.float64)
        sq2 = np.diag(g[:, :DIM]).copy()
        sk2 = np.diag(g[:, 2 * DIM:]).copy()
        rq = 1.0 / np.maximum(np.sqrt(sq2), 1e-12)
        rk = 1.0 / np.maximum(np.sqrt(sk2), 1e-12)
        attnb0 = np.empty((DIM, C))
        rkb = np.empty((DIM, C))
        for h in range(HEADS):
            attnb0[h * C:(h + 1) * C] = g[h * C:(h + 1) * C,
                                          DIM + h * C:DIM + (h + 1) * C]
            rkb[h * C:(h + 1) * C] = rk[h * C:(h + 1) * C]
        attn = attnb0 * rkb * rq[:, None] * temp128
        m = attn.max(axis=1, keepdims=True)
        e = np.exp(attn - m)
        attn0 = e / e.sum(axis=1, keepdims=True)
        rl = np.maximum(attn, 0.0)
        r2 = rl * rl
        erf = np.vectorize(math.erf)
        gl = 0.5 * r2 * (1.0 + erf(r2 / math.sqrt(2.0)))
        a1 = gl * r2
        ss = a1 @ attca2.T  # [128, 32]
        attnf = attn0 * (1.0 + ss[:, :C]) + ss[:, C:]
        bd = np.zeros((DIM, DIM))
        for h in range(HEADS):
            bd[h * C:(h + 1) * C, h * C:(h + 1) * C] = attnf[h * C:(h + 1) * C]
        mt = (bd.T @ projT).astype(f)  # [vd, o] = M^T
        wy = np.empty((DIM, 9 * DIM), dtype=f)
        for tap in range(9):
            wy[:, tap * DIM:(tap + 1) * DIM] = \
                wvr_f[:, tap * DIM:(tap + 1) * DIM].T @ mt
        wys.append(wy.astype(BF16))
    return wys


# ---------------------------------------------------------------- device ----

def build_kernels_into(holder, bass, bacc, mybir, tile):
    """Build both kernels, storing results/errors into `holder`.

    Used as a bare thread target (from the fixed-path copy of this module)
    so that no frame from an install-dependent path is on the stack while
    instructions record their debug info — that debug info is part of the
    BIR and hence of the compilation cache key."""
    try:
        holder["ncs"] = (_build_kernel_a(bass, bacc, mybir, tile),
                         _build_kernel_b(bass, bacc, mybir, tile))
    except BaseException as exc:  # noqa: BLE001
        holder["err"] = exc


def _build_kernel_a(bass, bacc, mybir, tile):
    """xpad, wqk -> per-core Gram partials [128, 384]."""
    nc = bacc.Bacc("TRN2", target_bir_lowering=False, debug=False,
                   num_devices=N_CORES)
    f32 = mybir.dt.float32
    bf16 = mybir.dt.bfloat16
    xpad = nc.dram_tensor("xpad", [DIM, ROWS + 2, W + 2], bf16, kind="ExternalInput").ap()
    wqk = nc.dram_tensor("wqk", [DIM, 9 * 2 * DIM], bf16, kind="ExternalInput").ap()
    gout = nc.dram_tensor("gout", [DIM, 3 * DIM], f32, kind="ExternalOutput").ap()

    with tile.TileContext(nc) as tc, ExitStack() as ctx:
        const = ctx.enter_context(tc.tile_pool(name="const", bufs=1))
        qkpool = ctx.enter_context(tc.tile_pool(name="qksb", bufs=4))
        gsb_pool = ctx.enter_context(tc.tile_pool(name="gsb", bufs=1))
        psqk = ctx.enter_context(tc.tile_pool(name="psqk", bufs=3, space="PSUM"))
        psg = ctx.enter_context(tc.tile_pool(name="psg", bufs=1, space="PSUM"))

        wqk_sb = const.tile([DIM, 9 * 2 * DIM], bf16)
        nc.sync.dma_start(wqk_sb[:], wqk)
        xsb = const.tile([DIM, ROWS + 2, W + 2], bf16)
        for lo, hi in [(0, 18), (18, 34), (34, 50), (50, ROWS + 2)]:
            nc.sync.dma_start(xsb[:, lo:hi, :], xpad[:, lo:hi, :])

        g1 = psg.tile([DIM, 2 * DIM], f32)   # q.q | q.k
        g2 = psg.tile([DIM, DIM], f32)       # k.k
        for ch in range(NCHUNK):
            r, wi = divmod(ch, 2)
            w0 = wi * 128
            pqk = psqk.tile([DIM, 2 * DIM], f32)
            for tap in range(9):
                dh, dw = divmod(tap, 3)
                nc.tensor.matmul(
                    pqk[:],
                    lhsT=xsb[:, r + dh, w0 + dw:w0 + dw + 128],
                    rhs=wqk_sb[:, tap * 2 * DIM:(tap + 1) * 2 * DIM],
                    start=(tap == 0), stop=(tap == 8),
                )
            qkt = qkpool.tile([DIM, 2 * DIM], f32)
            nc.vector.tensor_copy(out=qkt[:], in_=pqk[:])
            nc.tensor.matmul(g1[:], lhsT=qkt[:, :DIM], rhs=qkt[:],
                             start=(ch == 0), stop=(ch == NCHUNK - 1))
            nc.tensor.matmul(g2[:], lhsT=qkt[:, DIM:], rhs=qkt[:, DIM:],
                             start=(ch == 0), stop=(ch == NCHUNK - 1))

        gsb = gsb_pool.tile([DIM, 3 * DIM], f32)
        nc.vector.tensor_copy(out=gsb[:, :2 * DIM], in_=g1[:])
        nc.vector.tensor_copy(out=gsb[:, 2 * DIM:], in_=g2[:])
        nc.sync.dma_start(gout, gsb[:])
    nc.compile()
    return nc


def _build_kernel_b(bass, bacc, mybir, tile):
    """xpad, wy -> y via the folded 9-tap conv."""
    nc = bacc.Bacc("TRN2", target_bir_lowering=False, debug=False,
                   num_devices=N_CORES)
    bf16 = mybir.dt.bfloat16
    f32 = mybir.dt.float32
    xpad = nc.dram_tensor("xpad", [DIM, ROWS + 2, W + 2], bf16, kind="ExternalInput").ap()
    wy = nc.dram_tensor("wy", [DIM, 9 * DIM], bf16, kind="ExternalInput").ap()
    yout = nc.dram_tensor("yout", [DIM, L_CORE], bf16, kind="ExternalOutput").ap()

    with tile.TileContext(nc) as tc, ExitStack() as ctx:
        const = ctx.enter_context(tc.tile_pool(name="const", bufs=1))
        ypool = ctx.enter_context(tc.tile_pool(name="ysb", bufs=4))
        psy = ctx.enter_context(tc.tile_pool(name="psy", bufs=4, space="PSUM"))

        wy_sb = const.tile([DIM, 9 * DIM], bf16)
        nc.sync.dma_start(wy_sb[:], wy)
        xsb = const.tile([DIM, ROWS + 2, W + 2], bf16)
        for lo, hi in [(0, 18), (18, 34), (34, 50), (50, ROWS + 2)]:
            nc.sync.dma_start(xsb[:, lo:hi, :], xpad[:, lo:hi, :])

        for rp in range(NTILE):
            py = psy.tile([DIM, 512], f32)
            for tap in range(9):
                dh, dw = divmod(tap, 3)
                nc.tensor.matmul(
                    py[:],
                    lhsT=wy_sb[:, tap * DIM:(tap + 1) * DIM],
                    rhs=xsb[:, 2 * rp + dh:2 * rp + dh + 2, dw:dw + W],
                    start=(tap == 0), stop=(tap == 8),
                )
            y_sb = ypool.tile([DIM, 512], bf16)
            nc.vector.tensor_copy(out=y_sb[:], in_=py[:])
            nc.sync.dma_start(yout[:, rp * 512:(rp + 1) * 512], y_sb[:])
    nc.compile()
    return nc


def _builder_module():
    """Import this file's builders from a fixed path.

    The BIR embeds per-instruction debug info (source filename + line), so
    building from a varying directory changes the BIR bytes and misses the
    persistent compilation cache. Copying this file to a stable location
    and importing the builders from there keeps the cache key directory-
    independent.
    """
    import importlib.util
    import shutil

    os.makedirs(BUILD_DIR, exist_ok=True)
    fixed = os.path.join(BUILD_DIR, "bass_kernel_fixed.py")
    src = os.path.abspath(__file__)
    try:
        with open(src, "rb") as f:
            body = f.read()
        old = b""
        if os.path.exists(fixed):
            with open(fixed, "rb") as f:
                old = f.read()
        if old != body:
            shutil.copyfile(src, fixed)
        spec = importlib.util.spec_from_file_location("bass_kernel_fixed", fixed)
        mod = importlib.util.module_from_spec(spec)
        spec.loader.exec_module(mod)
        return mod
    except Exception:
        import sys
        return sys.modules[__name__]


def _run_device(xpads, wqk, wvr, proj_w, attca_w, temperature):
    os.makedirs(CACHE_DIR, exist_ok=True)
    os.environ.setdefault("JAX_COMPILATION_CACHE_DIR", CACHE_DIR)

    import jax
    jax.config.update("jax_compilation_cache_dir", CACHE_DIR)
    jax.config.update("jax_persistent_cache_min_entry_size_bytes", -1)
    jax.config.update("jax_persistent_cache_min_compile_time_secs", 0.0)

    import concourse.bass as bass
    import concourse.bacc as bacc
    import concourse.mybir as mybir
    import concourse.tile as tile
    from concourse import bass_utils

    import time as _time

    import threading

    core_ids = list(range(N_CORES))
    bmod = _builder_module()

    t0 = _time.perf_counter()
    # Build on a fresh thread whose every stack frame lives in the fixed
    # module (or the stdlib): instruction debug info captures the python
    # stack, lands in the BIR, and thus in the compilation cache key.
    holder = {}
    th = threading.Thread(target=bmod.build_kernels_into, name="bass-build",
                          args=(holder, bass, bacc, mybir, tile))
    th.start()
    th.join()
    if "err" in holder:
        raise holder["err"]
    nc_a, nc_b = holder["ncs"]
    t1 = _time.perf_counter()

    in_maps_a = [{"xpad": xpads[c], "wqk": wqk} for c in core_ids]
    res_a = bass_utils.run_bass_kernel_spmd(nc_a, in_maps_a, core_ids)
    t2 = _time.perf_counter()
    grams = [r["gout"] for r in res_a.results]

    wys = _attention_host(grams, wvr, proj_w, attca_w, temperature)

    t3 = _time.perf_counter()
    t4 = _time.perf_counter()
    in_maps_b = [{"xpad": xpads[c], "wy": wys[c // 4]} for c in core_ids]
    res_b = bass_utils.run_bass_kernel_spmd(nc_b, in_maps_b, core_ids)
    t5 = _time.perf_counter()

    LAST_TIMING["build_a_s"] = t1 - t0
    LAST_TIMING["run_a_s"] = t2 - t1
    LAST_TIMING["host_s"] = t3 - t2
    LAST_TIMING["build_b_s"] = t4 - t3
    LAST_TIMING["run_b_s"] = t5 - t4
    LAST_TIMING["kernel_a_ns"] = res_a.exec_time_ns
    LAST_TIMING["kernel_b_ns"] = res_b.exec_time_ns
    return [r["yout"] for r in res_b.results]


# ------------------------------------------------------------- emulation ----

def _emulate_device(xpads, wqk, wvr, proj_w, attca_w, temperature):
    """Numpy re-implementation of both device kernels."""
    f = np.float32
    wqk_f = wqk.astype(f)
    grams = []
    for core in range(N_CORES):
        xp = xpads[core].astype(f)  # [128, 66, 258]
        qk = np.zeros((2 * DIM, ROWS, W), dtype=f)
        for tap in range(9):
            dh, dw = divmod(tap, 3)
            xs = xp[:, dh:dh + ROWS, dw:dw + W]
            qk += np.einsum('io,ihw->ohw',
                            wqk_f[:, tap * 2 * DIM:(tap + 1) * 2 * DIM], xs)
        q = qk[:DIM].reshape(DIM, L_CORE)
        k = qk[DIM:].reshape(DIM, L_CORE)
        g = np.empty((DIM, 3 * DIM), dtype=f)
        g[:, :DIM] = q @ q.T
        g[:, DIM:2 * DIM] = q @ k.T
        g[:, 2 * DIM:] = k @ k.T
        grams.append(g)

    wys = _attention_host(grams, wvr, proj_w, attca_w, temperature)

    youts = []
    for core in range(N_CORES):
        wy = wys[core // 4].astype(f)
        xp = xpads[core].astype(f)
        y = np.zeros((DIM, ROWS, W), dtype=f)
        for tap in range(9):
            dh, dw = divmod(tap, 3)
            xs = xp[:, dh:dh + ROWS, dw:dw + W]
            y += np.einsum('io,ihw->ohw', wy[:, tap * DIM:(tap + 1) * DIM], xs)
        youts.append(y.reshape(DIM, L_CORE).astype(BF16))
    return youts


# ---------------------------------------------------------------- entry ----

def kernel(x, qkv_w, dw_w, proj_w, attca_w, temperature):
    x = np.asarray(x, dtype=np.float32)
    qkv_w = np.asarray(qkv_w, dtype=np.float32)
    dw_w = np.asarray(dw_w, dtype=np.float32)
    proj_w = np.asarray(proj_w, dtype=np.float32)
    attca_w = np.asarray(attca_w, dtype=np.float32)
    temperature = np.asarray(temperature, dtype=np.float32)

    xpads, wqk, wvr = _build_host_tensors(x, qkv_w, dw_w)

    if os.environ.get("KERNEL_EMULATE", "0") == "1":
        youts = _emulate_device(xpads, wqk, wvr, proj_w, attca_w, temperature)
    else:
        youts = _run_device(xpads, wqk, wvr, proj_w, attca_w, temperature)

    out = np.empty((B, DIM, H, W), dtype=np.float32)
    for core in range(N_CORES):
        b, quad = divmod(core, 4)
        r0 = quad * ROWS
        out[b, :, r0:r0 + ROWS, :] = \
            np.asarray(youts[core]).astype(np.float32).reshape(DIM, ROWS, W)
    return out
